# revision 13
# baseline (speedup 1.0000x reference)
"""Trainium2 Bass kernel: GroupNorm(32) + single-head self-attention block + residual.

fp8 (e4m3) DoubleRow edition with fused score projection AND fused output
projection. Per image:
    h  = group_norm(x)  (fp32 stats; normalized output quantized to fp8)
    sT[m, n] = h_m^T G h_n,  G = wk^T wq   (one Z = G h projection replaces
        separate Q and K projections; per-n bias terms cancel in softmax,
        exact when bq == 0 -- the general-bias fallback kernel keeps Q/K)
    p = exp(sT/sqrt(C) - 2)  (shift keeps exp < 240 = e4m3 max)        fp8
    v' = h^T Wvo^T,  Wvo = wo @ wv   (attention is linear after softmax so
        the output projection folds into V: out = P (h Wvo^T) + boP; the
        bv term rides through exactly because softmax rows sum to 1)    fp8
    y[c, n] = (sum_m v'[m,c] p[m,n]) / denom[n] + x[c, n]              fp32

G and Wvo are scaled x8 on host (lifts fp8 subnormals); the x8 cancels via
SCALE = 1/(8 sqrt(C)) in the exp and via an 8.0-valued colsum stationary.

All heavy matmuls run fp8e4 MatmulPerfMode.DoubleRow (contraction 256/instr,
~223ns per 512-free instr measured = the DR floor). Removing the separate
output projection cuts 16 of 120 DR matmuls per image.

Head/tail are DMA-descriptor-push bound: each dma_start costs ~650ns on its
queue engine, so x loads are 1 push per [128, 4KB] tile split across the two
HWDGE queues (Sync + ACT), weights pushed immediately after, and y stores go
out as [128, 4KB] per channel-tile.

Sharding: data-parallel over batch; 8 cores x 4 images each.
"""

import math
import os

import numpy as np
import ml_dtypes

import concourse.bass as bass
import concourse.tile as tile
from concourse import bacc, mybir
from concourse.bass_utils import run_bass_kernel_spmd

N_CORES = 8
B, C, H, W = 32, 512, 32, 32
HW = H * W                      # 1024 tokens
BL = B // N_CORES               # 4 images per core
NGRP = 32                       # groupnorm groups
GS = C // NGRP                  # 16 channels per group
EPS = 1e-5
P = 128
NT = C // P                     # 4 channel partition-tiles
KP = NT // 2                    # 2 channel k-tile pairs (DoubleRow)
MT = HW // P                    # 8 token partition-tiles
MP = MT // 2                    # 4 token k-tile pairs
FCH = 512                       # output free-dim chunk (one PSUM bank fp32)
NCH = HW // FCH                 # 2 free chunks per 1024
F32 = mybir.dt.float32
F8 = mybir.dt.float8e4
DR = mybir.MatmulPerfMode.DoubleRow
WSC = 8.0                       # host weight prescale (fp8 subnormal lift)
SCALE = 1.0 / (WSC * math.sqrt(C))
ESHIFT = -2.0                   # exp shift: keeps exp(s) under e4m3 max 240

ACT_EXP = mybir.ActivationFunctionType.Exp
ACT_LN = mybir.ActivationFunctionType.Ln
ACT_COPY = mybir.ActivationFunctionType.Copy
OP_ADD = mybir.AluOpType.add
OP_MULT = mybir.AluOpType.mult

LAST_EXEC_NS = None
_CACHED = {}


def _build_nc(fused):
    from contextlib import ExitStack

    esc = 1.0 / ((WSC if fused else 1.0) * math.sqrt(C))
    nc = bacc.Bacc("TRN2", target_bir_lowering=False, debug=False)

    x_d = nc.dram_tensor("x", [BL, C, HW], F32, kind="ExternalInput").ap()
    # paired fp8 weights: [p, k, i, o] = w.T[(2k+i)*128+p, o]
    # fused mode: the wq8 slot carries 8*G^T pairs for Z = G h (G = wk^T wq)
    wq8_d = nc.dram_tensor("wq8", [P, KP, 2, C], F8, kind="ExternalInput").ap()
    wk8_d = nc.dram_tensor("wk8", [P, KP, 2, C], F8, kind="ExternalInput").ap()
    # wv8 slot carries 8*Wvo^T pairs, Wvo = wo @ wv
    wv8_d = nc.dram_tensor("wv8", [P, KP, 2, C], F8, kind="ExternalInput").ap()
    e8_d = nc.dram_tensor("e8", [P, 2, P], F8, kind="ExternalInput").ap()
    # cols pack: [p, j, t]: j = 0 gw, 1 gb, 2 bq, 3 bk, 4 boP
    cols_d = nc.dram_tensor("cols", [P, 5, NT], F32, kind="ExternalInput").ap()
    gm_d = nc.dram_tensor("gm", [P, NT, NGRP], F32, kind="ExternalInput").ap()
    gmt_d = nc.dram_tensor("gmt", [NGRP, NT, P], F32, kind="ExternalInput").ap()
    y_d = nc.dram_tensor("y", [BL, C, HW], F32, kind="ExternalOutput").ap()

    x_r = x_d.rearrange("b (t p) n -> b t p n", p=P)
    y_r = y_d.rearrange("b (t p) n -> b t p n", p=P)

    ib = lambda k, d: int(os.environ.get(k, d))  # buf-count knobs for tuning
    with tile.TileContext(nc) as tc, ExitStack() as ctx:
        pool = lambda name, bufs, space="SBUF": ctx.enter_context(
            tc.tile_pool(name=name, bufs=bufs, space=space)
        )
        p_const = pool("const", 1)
        p_x = pool("x", ib("BUF_X", 16))
        p_X = pool("X", ib("BUF_XN", 4))
        p_z = pool("z", 4)
        p_kt = pool("kt", 4)
        p_v = pool("v", 2)
        p_e = pool("exp", ib("BUF_EXP", 8))
        p_recip = pool("recip", 2)
        p_out = pool("out", ib("BUF_OUT", 4))
        p_small = pool("small", 4)
        psum = pool("psum", ib("BUF_PSUM", 8), space="PSUM")

        def ps_tile(name, parts=P, free=FCH):
            return psum.tile([parts, free], F32, tag="u", name=name)

        # ---- x loads: one whole-tile [128, 4KB] DMA each; image 0 splits
        # its 4 pushes across the two HWDGE queues so the ~650ns descriptor
        # pushes overlap ----
        def emit_x(b, engs=None):
            xt = []
            for t in range(NT):
                xtile = p_x.tile([P, HW], F32, tag="x", name=f"x_{b}_{t}")
                eng = engs[t] if engs is not None else nc.sync
                eng.dma_start(out=xtile[:], in_=x_r[b, t])
                xt.append(xtile)
            return xt

        xt0 = emit_x(0)

        # ---- fp8 weights + consts ride the scalar HWDGE queue so image-0
        # x owns the sync queue's full issue slot sequence ----
        def load_w8(dram, tag, eng):
            t = p_const.tile([P, KP, 2, C], F8, tag=tag)
            eng.dma_start(out=t[:], in_=dram)
            return t

        wq8 = load_w8(wq8_d, "wq8", nc.scalar)   # Z weights (8G pairs) fused
        wv8 = load_w8(wv8_d, "wv8", nc.scalar)   # 8*Wvo pairs
        wk8 = None if fused else load_w8(wk8_d, "wk8", nc.scalar)

        # ---- small constants on the scalar queue ----
        gm_sb = p_const.tile([P, NT, NGRP], F32, tag="gm")
        nc.scalar.dma_start(out=gm_sb[:], in_=gm_d)
        gmt_sb = p_const.tile([NGRP, NT, P], F32, tag="gmt")
        nc.scalar.dma_start(out=gmt_sb[:], in_=gmt_d)
        cols_sb = p_const.tile([P, 5, NT], F32, tag="cols")
        nc.scalar.dma_start(out=cols_sb[:], in_=cols_d)
        e8_sb = p_const.tile([P, 2, P], F8, tag="e8")
        nc.scalar.dma_start(out=e8_sb[:], in_=e8_d)
        eps_sb = p_const.tile([P, 1], F32, tag="eps")
        nc.vector.memset(eps_sb[:], EPS)
        esh_sb = p_const.tile([P, 1], F32, tag="esh")
        nc.vector.memset(esh_sb[:], ESHIFT)

        gw_sb = cols_sb[:, 0, :]
        gb_sb = cols_sb[:, 1, :]
        bq_sb = cols_sb[:, 2, :]
        bk_sb = cols_sb[:, 3, :]
        boP_sb = cols_sb[:, 4, :]

        def emit_gn_stats(b, xt):
            """DVE-only per-tile stats: stat2 = [mean, var + mean^2].
            bn_stats is capped at 512 free, so 2 chunks per tile."""
            stat2s = []
            for t in range(NT):
                st = p_small.tile([P, NCH, 6], F32, tag="bnst")
                for i in range(NCH):
                    nc.vector.bn_stats(
                        out=st[:, i, :], in_=xt[t][:, i * FCH : (i + 1) * FCH]
                    )
                stat2 = p_small.tile([P, 2], F32, tag="stat2", name=f"stat2_{b}_{t}")
                nc.vector.bn_aggr(out=stat2[:], in_=st[:])
                # var += mean^2 in one fused op: (mean * mean) + var
                nc.vector.scalar_tensor_tensor(
                    out=stat2[:, 1:2], in0=stat2[:, 0:1], scalar=stat2[:, 0:1],
                    in1=stat2[:, 1:2], op0=OP_MULT, op1=OP_ADD,
                )
                stat2s.append(stat2)
            return xt, stat2s

        def emit_gn_reduce(b, state):
            """Group-reduce via PE; rstd = rsqrt(var+eps). Fused mode
            returns gmr = [rstd, mean*rstd] so the broadcast feeds the
            affine x*a - b directly (host guarantees gw==1, gb==0)."""
            xt, stat2s = state
            psg = ps_tile(f"psg_{b}", parts=NGRP, free=2)
            for t in range(NT):
                nc.tensor.matmul(
                    psg[:], gm_sb[:, t, :], stat2s[t][:],
                    start=(t == 0), stop=(t == NT - 1),
                )
            # gmr: [32 groups, (mean, rstd)]
            gmr = p_small.tile([NGRP, 2], F32, tag="gmr")
            nc.vector.tensor_scalar_mul(gmr[:, 0:1], psg[:, 0:1], 1.0 / GS)
            e2g = p_small.tile([NGRP, 1], F32, tag="e2g")
            nc.vector.tensor_scalar_mul(e2g[:], psg[:, 1:2], 1.0 / GS)
            m2g = p_small.tile([NGRP, 1], F32, tag="m2g")
            nc.vector.tensor_mul(m2g[:], gmr[:, 0:1], gmr[:, 0:1])
            if fused:
                # DVE-only rsqrt: group var is ~1 for standardized x (host
                # checks and falls back otherwise), so Newton from y0 =
                # 1.5 - 0.5*ve converges to <1e-6 for var in [0.5, 2].
                # Avoids ACT act-table loads entirely.
                ve = p_small.tile([NGRP, 1], F32, tag="ve")
                nc.vector.scalar_tensor_tensor(
                    out=ve[:], in0=e2g[:], scalar=m2g[:],
                    in1=eps_sb[0:NGRP, :], op0=mybir.AluOpType.subtract,
                    op1=OP_ADD,
                )
                y = p_small.tile([NGRP, 1], F32, tag="nwy")
                nc.vector.tensor_scalar(
                    out=y[:], in0=ve[:], scalar1=-0.5, scalar2=1.5,
                    op0=OP_MULT, op1=OP_ADD,
                )
                t1 = p_small.tile([NGRP, 1], F32, tag="nwt")
                nc.vector.tensor_mul(t1[:], ve[:], y[:])
                nc.vector.tensor_mul(t1[:], t1[:], y[:])
                nc.vector.tensor_scalar(
                    out=t1[:], in0=t1[:], scalar1=-0.5, scalar2=1.5,
                    op0=OP_MULT, op1=OP_ADD,
                )
                # pack gmr2 = [rstd, mean*rstd] for the trivial-affine path
                gmr2 = p_small.tile([NGRP, 2], F32, tag="gmr2")
                nc.vector.tensor_mul(gmr2[:, 0:1], y[:], t1[:])
                nc.vector.tensor_mul(gmr2[:, 1:2], gmr[:, 0:1], gmr2[:, 0:1])
                return xt, gmr2
            else:
                varg = p_small.tile([NGRP, 1], F32, tag="varg")
                nc.vector.tensor_sub(varg[:], e2g[:], m2g[:])
                lng = p_small.tile([NGRP, 1], F32, tag="lng")
                nc.scalar.activation(
                    out=lng[:], in_=varg[:], func=ACT_LN, bias=eps_sb[0:NGRP, :]
                )
                nc.scalar.activation(
                    out=gmr[:, 1:2], in_=lng[:], func=ACT_EXP, scale=-0.5
                )
            return xt, gmr

        def emit_gn_norm(b, state, head=False):
            """Broadcast group stats to channels, apply affine -> fp8 pairs.
            Head image alternates gpsimd/DVE per tile for latency; steady
            state keeps it all on gpsimd (DVE runs the evacs). Fused mode
            broadcasts [rstd, mean*rstd] into one PSUM bank, copies once to
            SBUF, and the affine is x*a - b (gw==1, gb==0 host-checked)."""
            xt, gmr = state
            X8 = [
                p_X.tile([P, 2, HW], F8, tag=f"X{k}", name=f"X_{b}_{k}")
                for k in range(KP)
            ]
            if fused:
                psb = ps_tile(f"psb_{b}", free=NT * 2)
                for t in range(NT):
                    nc.tensor.matmul(
                        psb[:, 2 * t : 2 * t + 2], gmt_sb[:, t, :], gmr[:],
                        start=True, stop=True,
                    )
                ab = p_small.tile([P, NT, 2], F32, tag="ab")
                nc.vector.tensor_copy(out=ab[:], in_=psb[:])
                for t in range(NT):
                    eng = (nc.vector if t % 2 else nc.gpsimd) if head else nc.gpsimd
                    eng.tensor_scalar(
                        out=X8[t // 2][:, t % 2, :], in0=xt[t][:],
                        scalar1=ab[:, t, 0:1], scalar2=ab[:, t, 1:2],
                        op0=OP_MULT, op1=mybir.AluOpType.subtract,
                    )
                return xt, X8
            for t in range(NT):
                psb = ps_tile(f"psb_{b}_{t}", free=2)
                nc.tensor.matmul(psb[:], gmt_sb[:, t, :], gmr[:], start=True, stop=True)
                acol = p_small.tile([P, 1], F32, tag="acol")
                nc.vector.tensor_mul(acol[:], psb[:, 1:2], gw_sb[:, t : t + 1])
                tmb = p_small.tile([P, 1], F32, tag="tmb")
                nc.vector.tensor_mul(tmb[:], psb[:, 0:1], acol[:])
                bcol = p_small.tile([P, 1], F32, tag="bcol")
                nc.vector.tensor_sub(bcol[:], gb_sb[:, t : t + 1], tmb[:])
                eng = (nc.vector if t % 2 else nc.gpsimd) if head else nc.gpsimd
                eng.tensor_scalar(
                    out=X8[t // 2][:, t % 2, :], in0=xt[t][:],
                    scalar1=acol[:], scalar2=bcol[:], op0=OP_MULT, op1=OP_ADD,
                )
            return xt, X8

        xt_pre = {1: emit_x(1)} if BL > 1 else {}
        gn_state = emit_gn_norm(
            0, emit_gn_reduce(0, emit_gn_stats(0, xt0)), head=True
        )

        # ---- per-image heavy phases ----
        for b in range(BL):
            xt, X8 = gn_state
            # prefetch x two images ahead so bn_stats never waits on DMA
            if b + 2 < BL:
                xt_pre[b + 2] = emit_x(b + 2)

            # channel-major projection [C, HW]; fused Z evacs ride on ACT
            # (Copy needs no act table), biased Q/K evacs use DVE
            def proj_cm(w8, bias_sb, out_pool, bname):
                dst = [
                    out_pool.tile(
                        [P, 2, HW], F8, tag=f"{bname}{k}", name=f"{bname}_{b}_{k}"
                    )
                    for k in range(KP)
                ]
                for ot in range(NT):
                    for nch in range(NCH):
                        ps = ps_tile(f"ps_{bname}_{b}_{ot}_{nch}")
                        for k in range(KP):
                            nc.tensor.matmul(
                                ps[:],
                                w8[:, k, :, ot * P : (ot + 1) * P],
                                X8[k][:, :, nch * FCH : (nch + 1) * FCH],
                                start=(k == 0),
                                stop=(k == KP - 1),
                                perf_mode=DR,
                            )
                        dslice = dst[ot // 2][:, ot % 2, nch * FCH : (nch + 1) * FCH]
                        if bias_sb is None:
                            nc.scalar.activation(
                                out=dslice, in_=ps[:], func=ACT_COPY, bias=0.0
                            )
                        else:
                            nc.vector.tensor_scalar(
                                out=dslice, in0=ps[:],
                                scalar1=bias_sb[:, ot : ot + 1], scalar2=None,
                                op0=OP_ADD,
                            )
                return dst

            if fused:
                Z8 = proj_cm(wq8, None, p_z, "z")
                s_stat, s_mov = X8, Z8
            else:
                QT8 = proj_cm(wq8, bq_sb, p_z, "q")
                KT8 = proj_cm(wk8, bk_sb, p_kt, "k")
                s_stat, s_mov = KT8, QT8

            # scores S^T[m, n] -> exp (fp8, shifted); per-m-pair tiles so PV
            # can start before the whole phase has evacuated
            E8 = [
                p_e.tile([P, 2, HW], F8, tag="exp", name=f"e_{b}_{j}")
                for j in range(MP)
            ]
            for mt in range(MT):
                for nch in range(NCH):
                    psS = ps_tile(f"ps_s_{b}_{mt}_{nch}")
                    for k in range(KP):
                        nc.tensor.matmul(
                            psS[:],
                            s_stat[k][:, :, mt * P : (mt + 1) * P],
                            s_mov[k][:, :, nch * FCH : (nch + 1) * FCH],
                            start=(k == 0),
                            stop=(k == KP - 1),
                            perf_mode=DR,
                        )
                    nc.scalar.activation(
                        out=E8[mt // 2][:, mt % 2, nch * FCH : (nch + 1) * FCH],
                        in_=psS[:], func=ACT_EXP, scale=esc, bias=esh_sb[:],
                    )

            # GN(b+1) stats+reduce: emitted after S so they queue on DVE
            # behind nothing (DVE idles during Z/S); the newton chain's
            # latency hides under the V matmuls
            reduce_next = (
                emit_gn_reduce(b + 1, emit_gn_stats(b + 1, xt_pre[b + 1]))
                if b + 1 < BL
                else None
            )

            # V' projection (token-major pairs); emitted after S so the PE
            # stays busy while ACT drains the exp backlog PV depends on
            V8 = p_v.tile([P, MP, 2, C], F8, tag="v", name=f"v_{b}")
            for mt in range(MT):
                ps = ps_tile(f"ps_v_{b}_{mt}")
                for k in range(KP):
                    nc.tensor.matmul(
                        ps[:],
                        X8[k][:, :, mt * P : (mt + 1) * P],
                        wv8[:, k, :, :],
                        start=(k == 0),
                        stop=(k == KP - 1),
                        perf_mode=DR,
                    )
                nc.vector.tensor_copy(out=V8[:, mt // 2, mt % 2, :], in_=ps[:])

            # GN(b+1) broadcast+affine: psb after V so it never stalls on the
            # newton chain; gpsimd affines overlap PV + evac
            gn_next = (
                emit_gn_norm(b + 1, reduce_next)
                if reduce_next is not None
                else None
            )

            # A^T[c, n] accumulated over m-pairs; y = A*recip + x fused into
            # the evac (output projection already folded into V'). colsum
            # uses an 8.0-valued stationary to cancel the host Wvo x8 scale.
            recip = p_recip.tile([P, HW], F32, tag="recip", name=f"recip_{b}")
            last = b == BL - 1
            for c2 in range(NT):
                oy = p_out.tile([P, HW], F32, tag="oy", name=f"oy_{b}_{c2}")
                psA = []
                for nch in range(NCH):
                    ps_at = ps_tile(f"ps_a_{b}_{c2}_{nch}")
                    for j in range(MP):
                        nc.tensor.matmul(
                            ps_at[:],
                            V8[:, j, :, c2 * P : (c2 + 1) * P],
                            E8[j][:, :, nch * FCH : (nch + 1) * FCH],
                            start=(j == 0),
                            stop=(j == MP - 1),
                            perf_mode=DR,
                        )
                    psA.append(ps_at)
                if c2 == 0:
                    for nch in range(NCH):
                        psc_t = ps_tile(f"psc_{b}_{nch}")
                        for j in range(MP):
                            nc.tensor.matmul(
                                psc_t[:],
                                e8_sb[:],
                                E8[j][:, :, nch * FCH : (nch + 1) * FCH],
                                start=(j == 0),
                                stop=(j == MP - 1),
                                perf_mode=DR,
                            )
                        nc.vector.reciprocal_approx_fast(
                            out=recip[:, nch * FCH : (nch + 1) * FCH],
                            in_=psc_t[:],
                        )
                for nch in range(NCH):
                    nc.vector.tensor_mul(
                        oy[:, nch * FCH : (nch + 1) * FCH],
                        psA[nch][:],
                        recip[:, nch * FCH : (nch + 1) * FCH],
                    )
                if fused:
                    # residual add in place; gpsimd steady-state (DVE runs
                    # stats/casts), and for the last image's drain the first
                    # half goes to gpsimd while DVE takes the final chunks
                    eng = nc.vector if (last and c2 >= NT // 2) else nc.gpsimd
                    eng.tensor_add(oy[:], oy[:], xt[c2][:])
                else:
                    for nch in range(NCH):
                        sl = slice(nch * FCH, (nch + 1) * FCH)
                        nc.vector.scalar_tensor_tensor(
                            out=oy[:, sl], in0=oy[:, sl],
                            scalar=boP_sb[:, c2 : c2 + 1],
                            in1=xt[c2][:, sl], op0=OP_ADD, op1=OP_ADD,
                        )
                # y store: one [128, 4KB] push; last image alternates queues
                eng = nc.scalar if (last and c2 % 2) else nc.sync
                eng.dma_start(out=y_r[b, c2], in_=oy[:])

            if gn_next is not None:
                gn_state = gn_next

    nc.compile()
    return nc


def _host_inputs(fused, x, gn_scale, gn_bias, wq, bq, wk, bk, wv, bv, wo, bo):
    f = lambda a: np.ascontiguousarray(np.asarray(a, dtype=np.float32))
    x = f(x).reshape(B, C, HW)
    boP = f(bo) + f(wo) @ f(bv)

    def pair8(wT):
        # [p, k, i, o] = wT[(2k+i)*128+p, o], quantized to e4m3
        t = np.ascontiguousarray(wT).reshape(KP, 2, P, C).transpose(2, 0, 1, 3)
        return np.ascontiguousarray(
            np.clip(t, -240, 240).astype(ml_dtypes.float8_e4m3)
        )

    if fused:
        G = f(wk).T @ f(wq)          # S^T[m,n] = h_m^T G h_n
        wq8 = pair8(WSC * G.T)       # stationary pairs of 8G
        wk8 = np.zeros((P, KP, 2, C), ml_dtypes.float8_e4m3)
    else:
        wq8 = pair8(f(wq).T)
        wk8 = pair8(f(wk).T)

    Wvo = f(wo) @ f(wv)              # out = P (h Wvo^T) + boP
    wv8 = pair8((WSC if fused else 1.0) * Wvo.T)

    gm = np.zeros((P, NT, NGRP), np.float32)
    gmt = np.zeros((NGRP, NT, P), np.float32)
    for t in range(NT):
        for p in range(P):
            g = (t * P + p) // GS
            gm[p, t, g] = 1.0
            gmt[g, t, p] = 1.0
    e8 = np.full((P, 2, P), WSC if fused else 1.0, ml_dtypes.float8_e4m3)

    colv = lambda a: f(a).reshape(NT, P).T            # [(t p)] -> [p, t]
    cols = np.ascontiguousarray(
        np.stack(
            [colv(gn_scale), colv(gn_bias), colv(bq), colv(bk), colv(boP)],
            axis=1,
        )
    )

    shared = {
        "wq8": wq8,
        "wk8": wk8,
        "wv8": wv8,
        "cols": cols,
        "gm": gm, "gmt": gmt, "e8": e8,
    }
    in_maps = []
    for i in range(N_CORES):
        m = dict(shared)
        m["x"] = np.ascontiguousarray(x[i * BL : (i + 1) * BL])
        in_maps.append(m)
    return in_maps


def kernel(x, gn_scale, gn_bias, wq, bq, wk, bk, wv, bv, wo, bo):
    global LAST_EXEC_NS
    assert x.shape == (B, C, H, W)
    # Z-fusion drops bq into softmax-invariant terms. The fused GN rstd uses
    # Newton-from-1 which needs roughly unit group variance, so oddly-scaled
    # x routes to the general kernel. In the fused kernel the exp scale
    # assumes the 8x host weight prescale; non-fused keeps unscaled weights.
    xs = np.asarray(x, np.float32).ravel()[::1031][:8192]
    fused = (
        not np.any(np.asarray(bq))
        and np.all(np.asarray(gn_scale) == 1.0)
        and not np.any(np.asarray(gn_bias))
        and 0.5 < float(xs.var()) < 2.0
    )
    if fused not in _CACHED:
        _CACHED[fused] = _build_nc(fused)
    in_maps = _host_inputs(
        fused, x, gn_scale, gn_bias, wq, bq, wk, bk, wv, bv, wo, bo
    )
    trace = os.environ.get("ATT_TRACE", "0") == "1"
    if not trace:
        # the NTFF trace path needs antenv.axon_hooks (shimmed only by our
        # test harness); make sure a stray BASS_TRACE can't drag us into it
        os.environ["BASS_NEVER_TRACE"] = "1"
    else:
        os.environ.pop("BASS_NEVER_TRACE", None)
    kwargs = {}
    tdir = os.environ.get("ATT_TRACE_DIR")
    if tdir:
        kwargs["tmpdir"] = tdir
    res = run_bass_kernel_spmd(
        _CACHED[fused], in_maps, core_ids=list(range(N_CORES)), trace=trace, **kwargs
    )
    LAST_EXEC_NS = res.exec_time_ns
    y = np.concatenate([res.results[i]["y"] for i in range(N_CORES)], axis=0)
    return y.reshape(B, C, H, W).astype(np.float32)


# revision 15
# speedup vs baseline: 1.9181x; 1.9181x over previous
"""Trainium2 Bass kernel: GroupNorm(32) + single-head self-attention block + residual.

fp8 (e4m3) DoubleRow edition with fused score projection AND fused output
projection. Per image:
    h  = group_norm(x)  (fp32 stats; normalized output quantized to fp8)
    sT[m, n] = h_m^T G h_n,  G = wk^T wq   (one Z = G h projection replaces
        separate Q and K projections; per-n bias terms cancel in softmax,
        exact when bq == 0 -- the general-bias fallback kernel keeps Q/K)
    p = exp(sT/sqrt(C) - 2)  (shift keeps exp < 240 = e4m3 max)        fp8
    v' = h^T Wvo^T,  Wvo = wo @ wv   (attention is linear after softmax so
        the output projection folds into V: out = P (h Wvo^T) + boP; the
        bv term rides through exactly because softmax rows sum to 1)    fp8
    y[c, n] = (sum_m v'[m,c] p[m,n]) / denom[n] + x[c, n]              fp32

G and Wvo are scaled x8 on host (lifts fp8 subnormals); the x8 cancels via
SCALE = 1/(8 sqrt(C)) in the exp and via an 8.0-valued colsum stationary.

All heavy matmuls run fp8e4 MatmulPerfMode.DoubleRow (contraction 256/instr,
~223ns per 512-free instr measured = the DR floor). Removing the separate
output projection cuts 16 of 120 DR matmuls per image.

Head/tail are DMA-descriptor-push bound: each dma_start costs ~650ns on its
queue engine, so x loads are 1 push per [128, 4KB] tile split across the two
HWDGE queues (Sync + ACT), weights pushed immediately after, and y stores go
out as [128, 4KB] per channel-tile.

Sharding: data-parallel over batch; 8 cores x 4 images each.
"""

import math
import os

import numpy as np
import ml_dtypes

import concourse.bass as bass
import concourse.tile as tile
from concourse import bacc, mybir
from concourse.bass_utils import run_bass_kernel_spmd

N_CORES = 8
B, C, H, W = 32, 512, 32, 32
HW = H * W                      # 1024 tokens
BL = B // N_CORES               # 4 images per core
NGRP = 32                       # groupnorm groups
GS = C // NGRP                  # 16 channels per group
EPS = 1e-5
P = 128
NT = C // P                     # 4 channel partition-tiles
KP = NT // 2                    # 2 channel k-tile pairs (DoubleRow)
MT = HW // P                    # 8 token partition-tiles
MP = MT // 2                    # 4 token k-tile pairs
FCH = 512                       # output free-dim chunk (one PSUM bank fp32)
NCH = HW // FCH                 # 2 free chunks per 1024
F32 = mybir.dt.float32
F8 = mybir.dt.float8e4
DR = mybir.MatmulPerfMode.DoubleRow
WSC = 8.0                       # host weight prescale (fp8 subnormal lift)
SCALE = 1.0 / (WSC * math.sqrt(C))
ESHIFT = -2.0                   # exp shift: keeps exp(s) under e4m3 max 240

ACT_EXP = mybir.ActivationFunctionType.Exp
ACT_LN = mybir.ActivationFunctionType.Ln
ACT_COPY = mybir.ActivationFunctionType.Copy
OP_ADD = mybir.AluOpType.add
OP_MULT = mybir.AluOpType.mult

LAST_EXEC_NS = None
_CACHED = {}


def _build_nc(fused):
    from contextlib import ExitStack

    esc = 1.0 / ((WSC if fused else 1.0) * math.sqrt(C))
    nc = bacc.Bacc("TRN2", target_bir_lowering=False, debug=False)

    x_d = nc.dram_tensor("x", [BL, C, HW], F32, kind="ExternalInput").ap()
    # paired fp8 weights: [p, k, i, o] = w.T[(2k+i)*128+p, o]
    # fused mode: the wq8 slot carries 8*G^T pairs for Z = G h (G = wk^T wq)
    wq8_d = nc.dram_tensor("wq8", [P, KP, 2, C], F8, kind="ExternalInput").ap()
    wk8_d = nc.dram_tensor("wk8", [P, KP, 2, C], F8, kind="ExternalInput").ap()
    # wv8 slot carries 8*Wvo^T pairs, Wvo = wo @ wv
    wv8_d = nc.dram_tensor("wv8", [P, KP, 2, C], F8, kind="ExternalInput").ap()
    e8_d = nc.dram_tensor("e8", [P, 2, P], F8, kind="ExternalInput").ap()
    # cols pack: [p, j, t]: j = 0 gw, 1 gb, 2 bq, 3 bk, 4 boP
    cols_d = nc.dram_tensor("cols", [P, 5, NT], F32, kind="ExternalInput").ap()
    gm_d = nc.dram_tensor("gm", [P, NT, NGRP], F32, kind="ExternalInput").ap()
    gmt_d = nc.dram_tensor("gmt", [NGRP, NT, P], F32, kind="ExternalInput").ap()
    y_d = nc.dram_tensor("y", [BL, C, HW], F32, kind="ExternalOutput").ap()

    x_r = x_d.rearrange("b (t p) n -> b t p n", p=P)
    y_r = y_d.rearrange("b (t p) n -> b t p n", p=P)

    ib = lambda k, d: int(os.environ.get(k, d))  # buf-count knobs for tuning
    with tile.TileContext(nc) as tc, ExitStack() as ctx:
        pool = lambda name, bufs, space="SBUF": ctx.enter_context(
            tc.tile_pool(name=name, bufs=bufs, space=space)
        )
        p_const = pool("const", 1)
        p_x = pool("x", ib("BUF_X", 16))
        p_X = pool("X", ib("BUF_XN", 4))
        p_z = pool("z", 4)
        p_kt = pool("kt", 4)
        p_v = pool("v", 2)
        p_e = pool("exp", ib("BUF_EXP", 8))
        p_recip = pool("recip", 2)
        p_out = pool("out", ib("BUF_OUT", 4))
        p_small = pool("small", 4)
        psum = pool("psum", ib("BUF_PSUM", 8), space="PSUM")

        def ps_tile(name, parts=P, free=FCH):
            return psum.tile([parts, free], F32, tag="u", name=name)

        # ---- x loads: one whole-tile [128, 4KB] DMA each; image 0 splits
        # its 4 pushes across the two HWDGE queues so the ~650ns descriptor
        # pushes overlap ----
        def emit_x(b, engs=None):
            xt = []
            for t in range(NT):
                xtile = p_x.tile([P, HW], F32, tag="x", name=f"x_{b}_{t}")
                eng = engs[t] if engs is not None else nc.sync
                eng.dma_start(out=xtile[:], in_=x_r[b, t])
                xt.append(xtile)
            return xt

        xt0 = emit_x(0)

        # ---- fp8 weights + consts ride the scalar HWDGE queue so image-0
        # x owns the sync queue's full issue slot sequence ----
        def load_w8(dram, tag, eng):
            t = p_const.tile([P, KP, 2, C], F8, tag=tag)
            eng.dma_start(out=t[:], in_=dram)
            return t

        wq8 = load_w8(wq8_d, "wq8", nc.scalar)   # Z weights (8G pairs) fused
        wv8 = load_w8(wv8_d, "wv8", nc.scalar)   # 8*Wvo pairs
        wk8 = None if fused else load_w8(wk8_d, "wk8", nc.scalar)

        # ---- small constants on the scalar queue ----
        gm_sb = p_const.tile([P, NT, NGRP], F32, tag="gm")
        nc.scalar.dma_start(out=gm_sb[:], in_=gm_d)
        gmt_sb = p_const.tile([NGRP, NT, P], F32, tag="gmt")
        nc.scalar.dma_start(out=gmt_sb[:], in_=gmt_d)
        cols_sb = p_const.tile([P, 5, NT], F32, tag="cols")
        nc.scalar.dma_start(out=cols_sb[:], in_=cols_d)
        e8_sb = p_const.tile([P, 2, P], F8, tag="e8")
        nc.scalar.dma_start(out=e8_sb[:], in_=e8_d)
        eps_sb = p_const.tile([P, 1], F32, tag="eps")
        nc.vector.memset(eps_sb[:], EPS)
        esh_sb = p_const.tile([P, 1], F32, tag="esh")
        nc.vector.memset(esh_sb[:], ESHIFT)

        gw_sb = cols_sb[:, 0, :]
        gb_sb = cols_sb[:, 1, :]
        bq_sb = cols_sb[:, 2, :]
        bk_sb = cols_sb[:, 3, :]
        boP_sb = cols_sb[:, 4, :]

        def emit_gn_stats(b, xt):
            """DVE-only per-tile stats: stat2 = [mean, var + mean^2].
            bn_stats is capped at 512 free, so 2 chunks per tile."""
            stat2s = []
            for t in range(NT):
                st = p_small.tile([P, NCH, 6], F32, tag="bnst")
                for i in range(NCH):
                    nc.vector.bn_stats(
                        out=st[:, i, :], in_=xt[t][:, i * FCH : (i + 1) * FCH]
                    )
                stat2 = p_small.tile([P, 2], F32, tag="stat2", name=f"stat2_{b}_{t}")
                nc.vector.bn_aggr(out=stat2[:], in_=st[:])
                # var += mean^2 in one fused op: (mean * mean) + var
                nc.vector.scalar_tensor_tensor(
                    out=stat2[:, 1:2], in0=stat2[:, 0:1], scalar=stat2[:, 0:1],
                    in1=stat2[:, 1:2], op0=OP_MULT, op1=OP_ADD,
                )
                stat2s.append(stat2)
            return xt, stat2s

        def emit_gn_reduce(b, state):
            """Group-reduce via PE; rstd = rsqrt(var+eps). Fused mode
            returns gmr = [rstd, mean*rstd] so the broadcast feeds the
            affine x*a - b directly (host guarantees gw==1, gb==0)."""
            xt, stat2s = state
            psg = ps_tile(f"psg_{b}", parts=NGRP, free=2)
            for t in range(NT):
                nc.tensor.matmul(
                    psg[:], gm_sb[:, t, :], stat2s[t][:],
                    start=(t == 0), stop=(t == NT - 1),
                )
            # gmr: [32 groups, (mean, rstd)]; fused keeps -mean so every
            # downstream op stays on the fast (MULT, ADD) ucode path —
            # subtract variants of tensor_scalar/STT are ~10-70x slower
            gmr = p_small.tile([NGRP, 2], F32, tag="gmr")
            nc.vector.tensor_scalar_mul(
                gmr[:, 0:1], psg[:, 0:1], (-1.0 if fused else 1.0) / GS
            )
            e2g = p_small.tile([NGRP, 1], F32, tag="e2g")
            nc.vector.tensor_scalar_mul(e2g[:], psg[:, 1:2], 1.0 / GS)
            m2g = p_small.tile([NGRP, 1], F32, tag="m2g")
            nc.vector.tensor_mul(m2g[:], gmr[:, 0:1], gmr[:, 0:1])
            if fused:
                # DVE-only rsqrt: group var is ~1 for standardized x (host
                # checks and falls back otherwise), so Newton from y0 =
                # 1.5 - 0.5*ve converges to <1e-6 for var in [0.5, 2].
                # Avoids ACT act-table loads entirely. eps = 1e-5 is
                # dropped: var ~ 1 so it shifts rstd by ~5e-6 relative.
                ve = p_small.tile([NGRP, 1], F32, tag="ve")
                nc.vector.tensor_scalar(
                    out=ve[:], in0=m2g[:], scalar1=-1.0, scalar2=e2g[:],
                    op0=OP_MULT, op1=OP_ADD,
                )
                y = p_small.tile([NGRP, 1], F32, tag="nwy")
                nc.vector.tensor_scalar(
                    out=y[:], in0=ve[:], scalar1=-0.5, scalar2=1.5,
                    op0=OP_MULT, op1=OP_ADD,
                )
                t1 = p_small.tile([NGRP, 1], F32, tag="nwt")
                nc.vector.tensor_mul(t1[:], ve[:], y[:])
                nc.vector.tensor_mul(t1[:], t1[:], y[:])
                nc.vector.tensor_scalar(
                    out=t1[:], in0=t1[:], scalar1=-0.5, scalar2=1.5,
                    op0=OP_MULT, op1=OP_ADD,
                )
                # pack gmr2 = [rstd, -mean*rstd] for the affine x*a + b
                gmr2 = p_small.tile([NGRP, 2], F32, tag="gmr2")
                nc.vector.tensor_mul(gmr2[:, 0:1], y[:], t1[:])
                nc.vector.tensor_mul(gmr2[:, 1:2], gmr[:, 0:1], gmr2[:, 0:1])
                return xt, gmr2
            else:
                varg = p_small.tile([NGRP, 1], F32, tag="varg")
                nc.vector.tensor_sub(varg[:], e2g[:], m2g[:])
                lng = p_small.tile([NGRP, 1], F32, tag="lng")
                nc.scalar.activation(
                    out=lng[:], in_=varg[:], func=ACT_LN, bias=eps_sb[0:NGRP, :]
                )
                nc.scalar.activation(
                    out=gmr[:, 1:2], in_=lng[:], func=ACT_EXP, scale=-0.5
                )
            return xt, gmr

        def emit_gn_norm(b, state, head=False):
            """Broadcast group stats to channels, apply affine -> fp8 pairs.
            Head image alternates gpsimd/DVE per tile for latency; steady
            state keeps it all on gpsimd (DVE runs the evacs). Fused mode
            broadcasts [rstd, mean*rstd] into one PSUM bank, copies once to
            SBUF, and the affine is x*a - b (gw==1, gb==0 host-checked)."""
            xt, gmr = state
            X8 = [
                p_X.tile([P, 2, HW], F8, tag=f"X{k}", name=f"X_{b}_{k}")
                for k in range(KP)
            ]
            if fused:
                psb = ps_tile(f"psb_{b}", free=NT * 2)
                for t in range(NT):
                    nc.tensor.matmul(
                        psb[:, 2 * t : 2 * t + 2], gmt_sb[:, t, :], gmr[:],
                        start=True, stop=True,
                    )
                ab = p_small.tile([P, NT, 2], F32, tag="ab")
                nc.vector.tensor_copy(out=ab[:], in_=psb[:])
                for t in range(NT):
                    eng = (nc.vector if t % 2 else nc.gpsimd) if head else nc.gpsimd
                    eng.tensor_scalar(
                        out=X8[t // 2][:, t % 2, :], in0=xt[t][:],
                        scalar1=ab[:, t, 0:1], scalar2=ab[:, t, 1:2],
                        op0=OP_MULT, op1=OP_ADD,
                    )
                return xt, X8
            for t in range(NT):
                psb = ps_tile(f"psb_{b}_{t}", free=2)
                nc.tensor.matmul(psb[:], gmt_sb[:, t, :], gmr[:], start=True, stop=True)
                acol = p_small.tile([P, 1], F32, tag="acol")
                nc.vector.tensor_mul(acol[:], psb[:, 1:2], gw_sb[:, t : t + 1])
                tmb = p_small.tile([P, 1], F32, tag="tmb")
                nc.vector.tensor_mul(tmb[:], psb[:, 0:1], acol[:])
                bcol = p_small.tile([P, 1], F32, tag="bcol")
                nc.vector.tensor_sub(bcol[:], gb_sb[:, t : t + 1], tmb[:])
                eng = (nc.vector if t % 2 else nc.gpsimd) if head else nc.gpsimd
                eng.tensor_scalar(
                    out=X8[t // 2][:, t % 2, :], in0=xt[t][:],
                    scalar1=acol[:], scalar2=bcol[:], op0=OP_MULT, op1=OP_ADD,
                )
            return xt, X8

        xt_pre = {1: emit_x(1)} if BL > 1 else {}
        gn_state = emit_gn_norm(
            0, emit_gn_reduce(0, emit_gn_stats(0, xt0)), head=True
        )

        # ---- per-image heavy phases ----
        for b in range(BL):
            xt, X8 = gn_state
            # prefetch x two images ahead so bn_stats never waits on DMA
            if b + 2 < BL:
                xt_pre[b + 2] = emit_x(b + 2)

            # channel-major projection [C, HW]; fused Z evacs ride on ACT
            # (Copy needs no act table), biased Q/K evacs use DVE
            def proj_cm(w8, bias_sb, out_pool, bname):
                dst = [
                    out_pool.tile(
                        [P, 2, HW], F8, tag=f"{bname}{k}", name=f"{bname}_{b}_{k}"
                    )
                    for k in range(KP)
                ]
                for ot in range(NT):
                    for nch in range(NCH):
                        ps = ps_tile(f"ps_{bname}_{b}_{ot}_{nch}")
                        for k in range(KP):
                            nc.tensor.matmul(
                                ps[:],
                                w8[:, k, :, ot * P : (ot + 1) * P],
                                X8[k][:, :, nch * FCH : (nch + 1) * FCH],
                                start=(k == 0),
                                stop=(k == KP - 1),
                                perf_mode=DR,
                            )
                        dslice = dst[ot // 2][:, ot % 2, nch * FCH : (nch + 1) * FCH]
                        if bias_sb is None:
                            nc.scalar.activation(
                                out=dslice, in_=ps[:], func=ACT_COPY, bias=0.0
                            )
                        else:
                            nc.vector.tensor_scalar(
                                out=dslice, in0=ps[:],
                                scalar1=bias_sb[:, ot : ot + 1], scalar2=None,
                                op0=OP_ADD,
                            )
                return dst

            if fused:
                Z8 = proj_cm(wq8, None, p_z, "z")
                s_stat, s_mov = X8, Z8
            else:
                QT8 = proj_cm(wq8, bq_sb, p_z, "q")
                KT8 = proj_cm(wk8, bk_sb, p_kt, "k")
                s_stat, s_mov = KT8, QT8

            # scores S^T[m, n] -> exp (fp8, shifted); per-m-pair tiles so PV
            # can start before the whole phase has evacuated
            E8 = [
                p_e.tile([P, 2, HW], F8, tag="exp", name=f"e_{b}_{j}")
                for j in range(MP)
            ]
            for mt in range(MT):
                for nch in range(NCH):
                    psS = ps_tile(f"ps_s_{b}_{mt}_{nch}")
                    for k in range(KP):
                        nc.tensor.matmul(
                            psS[:],
                            s_stat[k][:, :, mt * P : (mt + 1) * P],
                            s_mov[k][:, :, nch * FCH : (nch + 1) * FCH],
                            start=(k == 0),
                            stop=(k == KP - 1),
                            perf_mode=DR,
                        )
                    nc.scalar.activation(
                        out=E8[mt // 2][:, mt % 2, nch * FCH : (nch + 1) * FCH],
                        in_=psS[:], func=ACT_EXP, scale=esc, bias=esh_sb[:],
                    )

            # GN(b+1) stats+reduce: emitted after S so they queue on DVE
            # behind nothing (DVE idles during Z/S); the newton chain's
            # latency hides under the V matmuls
            reduce_next = (
                emit_gn_reduce(b + 1, emit_gn_stats(b + 1, xt_pre[b + 1]))
                if b + 1 < BL
                else None
            )

            # V' projection (token-major pairs); emitted after S so the PE
            # stays busy while ACT drains the exp backlog PV depends on
            V8 = p_v.tile([P, MP, 2, C], F8, tag="v", name=f"v_{b}")
            for mt in range(MT):
                ps = ps_tile(f"ps_v_{b}_{mt}")
                for k in range(KP):
                    nc.tensor.matmul(
                        ps[:],
                        X8[k][:, :, mt * P : (mt + 1) * P],
                        wv8[:, k, :, :],
                        start=(k == 0),
                        stop=(k == KP - 1),
                        perf_mode=DR,
                    )
                nc.vector.tensor_copy(out=V8[:, mt // 2, mt % 2, :], in_=ps[:])

            # GN(b+1) broadcast+affine: psb after V so it never stalls on the
            # newton chain; gpsimd affines overlap PV + evac
            gn_next = (
                emit_gn_norm(b + 1, reduce_next)
                if reduce_next is not None
                else None
            )

            # A^T[c, n] accumulated over m-pairs; y = A*recip + x fused into
            # the evac (output projection already folded into V'). colsum
            # uses an 8.0-valued stationary to cancel the host Wvo x8 scale.
            recip = p_recip.tile([P, HW], F32, tag="recip", name=f"recip_{b}")
            last = b == BL - 1
            for c2 in range(NT):
                oy = p_out.tile([P, HW], F32, tag="oy", name=f"oy_{b}_{c2}")
                psA = []
                for nch in range(NCH):
                    ps_at = ps_tile(f"ps_a_{b}_{c2}_{nch}")
                    for j in range(MP):
                        nc.tensor.matmul(
                            ps_at[:],
                            V8[:, j, :, c2 * P : (c2 + 1) * P],
                            E8[j][:, :, nch * FCH : (nch + 1) * FCH],
                            start=(j == 0),
                            stop=(j == MP - 1),
                            perf_mode=DR,
                        )
                    psA.append(ps_at)
                if c2 == 0:
                    for nch in range(NCH):
                        psc_t = ps_tile(f"psc_{b}_{nch}")
                        for j in range(MP):
                            nc.tensor.matmul(
                                psc_t[:],
                                e8_sb[:],
                                E8[j][:, :, nch * FCH : (nch + 1) * FCH],
                                start=(j == 0),
                                stop=(j == MP - 1),
                                perf_mode=DR,
                            )
                        nc.vector.reciprocal_approx_fast(
                            out=recip[:, nch * FCH : (nch + 1) * FCH],
                            in_=psc_t[:],
                        )
                for nch in range(NCH):
                    nc.vector.tensor_mul(
                        oy[:, nch * FCH : (nch + 1) * FCH],
                        psA[nch][:],
                        recip[:, nch * FCH : (nch + 1) * FCH],
                    )
                if fused:
                    # residual add in place; gpsimd steady-state (DVE runs
                    # stats/casts), and for the last image's drain the first
                    # half goes to gpsimd while DVE takes the final chunks
                    eng = nc.vector if (last and c2 >= NT // 2) else nc.gpsimd
                    eng.tensor_add(oy[:], oy[:], xt[c2][:])
                else:
                    for nch in range(NCH):
                        sl = slice(nch * FCH, (nch + 1) * FCH)
                        nc.vector.scalar_tensor_tensor(
                            out=oy[:, sl], in0=oy[:, sl],
                            scalar=boP_sb[:, c2 : c2 + 1],
                            in1=xt[c2][:, sl], op0=OP_ADD, op1=OP_ADD,
                        )
                # y store: one [128, 4KB] push; last image alternates queues
                eng = nc.scalar if (last and c2 % 2) else nc.sync
                eng.dma_start(out=y_r[b, c2], in_=oy[:])

            if gn_next is not None:
                gn_state = gn_next

    nc.compile()
    return nc


def _host_inputs(fused, x, gn_scale, gn_bias, wq, bq, wk, bk, wv, bv, wo, bo):
    f = lambda a: np.ascontiguousarray(np.asarray(a, dtype=np.float32))
    x = f(x).reshape(B, C, HW)
    boP = f(bo) + f(wo) @ f(bv)

    def pair8(wT):
        # [p, k, i, o] = wT[(2k+i)*128+p, o], quantized to e4m3
        t = np.ascontiguousarray(wT).reshape(KP, 2, P, C).transpose(2, 0, 1, 3)
        return np.ascontiguousarray(
            np.clip(t, -240, 240).astype(ml_dtypes.float8_e4m3)
        )

    if fused:
        G = f(wk).T @ f(wq)          # S^T[m,n] = h_m^T G h_n
        wq8 = pair8(WSC * G.T)       # stationary pairs of 8G
        wk8 = np.zeros((P, KP, 2, C), ml_dtypes.float8_e4m3)
    else:
        wq8 = pair8(f(wq).T)
        wk8 = pair8(f(wk).T)

    Wvo = f(wo) @ f(wv)              # out = P (h Wvo^T) + boP
    wv8 = pair8((WSC if fused else 1.0) * Wvo.T)

    gm = np.zeros((P, NT, NGRP), np.float32)
    gmt = np.zeros((NGRP, NT, P), np.float32)
    for t in range(NT):
        for p in range(P):
            g = (t * P + p) // GS
            gm[p, t, g] = 1.0
            gmt[g, t, p] = 1.0
    e8 = np.full((P, 2, P), WSC if fused else 1.0, ml_dtypes.float8_e4m3)

    colv = lambda a: f(a).reshape(NT, P).T            # [(t p)] -> [p, t]
    cols = np.ascontiguousarray(
        np.stack(
            [colv(gn_scale), colv(gn_bias), colv(bq), colv(bk), colv(boP)],
            axis=1,
        )
    )

    shared = {
        "wq8": wq8,
        "wk8": wk8,
        "wv8": wv8,
        "cols": cols,
        "gm": gm, "gmt": gmt, "e8": e8,
    }
    in_maps = []
    for i in range(N_CORES):
        m = dict(shared)
        m["x"] = np.ascontiguousarray(x[i * BL : (i + 1) * BL])
        in_maps.append(m)
    return in_maps


def kernel(x, gn_scale, gn_bias, wq, bq, wk, bk, wv, bv, wo, bo):
    global LAST_EXEC_NS
    assert x.shape == (B, C, H, W)
    # Z-fusion drops bq into softmax-invariant terms. The fused GN rstd uses
    # Newton-from-1 which needs roughly unit group variance, so oddly-scaled
    # x routes to the general kernel. In the fused kernel the exp scale
    # assumes the 8x host weight prescale; non-fused keeps unscaled weights.
    xs = np.asarray(x, np.float32).ravel()[::1031][:8192]
    fused = (
        not np.any(np.asarray(bq))
        and np.all(np.asarray(gn_scale) == 1.0)
        and not np.any(np.asarray(gn_bias))
        and 0.5 < float(xs.var()) < 2.0
    )
    if fused not in _CACHED:
        _CACHED[fused] = _build_nc(fused)
    in_maps = _host_inputs(
        fused, x, gn_scale, gn_bias, wq, bq, wk, bk, wv, bv, wo, bo
    )
    trace = os.environ.get("ATT_TRACE", "0") == "1"
    if not trace:
        # the NTFF trace path needs antenv.axon_hooks (shimmed only by our
        # test harness); make sure a stray BASS_TRACE can't drag us into it
        os.environ["BASS_NEVER_TRACE"] = "1"
    else:
        os.environ.pop("BASS_NEVER_TRACE", None)
    kwargs = {}
    tdir = os.environ.get("ATT_TRACE_DIR")
    if tdir:
        kwargs["tmpdir"] = tdir
    res = run_bass_kernel_spmd(
        _CACHED[fused], in_maps, core_ids=list(range(N_CORES)), trace=trace, **kwargs
    )
    LAST_EXEC_NS = res.exec_time_ns
    y = np.concatenate([res.results[i]["y"] for i in range(N_CORES)], axis=0)
    return y.reshape(B, C, H, W).astype(np.float32)


# revision 22
# speedup vs baseline: 2.2975x; 1.1978x over previous
"""Trainium2 Bass kernel: GroupNorm(32) + single-head self-attention block + residual.

fp8 (e4m3) DoubleRow edition with fused score projection AND fused output
projection. Per image:
    h  = group_norm(x)  (fp32 stats; normalized output quantized to fp8)
    sT[m, n] = h_m^T G h_n,  G = wk^T wq   (one Z = G h projection replaces
        separate Q and K projections; per-n bias terms cancel in softmax,
        exact when bq == 0 -- the general-bias fallback kernel keeps Q/K)
    p = exp(sT/sqrt(C) - 2)  (shift keeps exp < 240 = e4m3 max)        fp8
    v' = h^T Wvo^T,  Wvo = wo @ wv   (attention is linear after softmax so
        the output projection folds into V: out = P (h Wvo^T) + boP; the
        bv term rides through exactly because softmax rows sum to 1)    fp8
    y[c, n] = (sum_m v'[m,c] p[m,n]) / denom[n] + x[c, n]              fp32

G and Wvo are scaled x8 on host (lifts fp8 subnormals); the x8 cancels via
SCALE = 1/(8 sqrt(C)) in the exp and via an 8.0-valued colsum stationary.

All heavy matmuls run fp8e4 MatmulPerfMode.DoubleRow (contraction 256/instr,
~223ns per 512-free instr measured = the DR floor). Removing the separate
output projection cuts 16 of 120 DR matmuls per image.

Head/tail are DMA-descriptor-push bound: each dma_start costs ~650ns on its
queue engine, so x loads are 1 push per [128, 4KB] tile split across the two
HWDGE queues (Sync + ACT), weights pushed immediately after, and y stores go
out as [128, 4KB] per channel-tile.

Sharding: data-parallel over batch; 8 cores x 4 images each.
"""

import math
import os

import numpy as np
import ml_dtypes

import concourse.bass as bass
import concourse.tile as tile
from concourse import bacc, mybir
from concourse.bass_utils import run_bass_kernel_spmd

N_CORES = 8
B, C, H, W = 32, 512, 32, 32
HW = H * W                      # 1024 tokens
BL = B // N_CORES               # 4 images per core
NGRP = 32                       # groupnorm groups
GS = C // NGRP                  # 16 channels per group
EPS = 1e-5
P = 128
NT = C // P                     # 4 channel partition-tiles
KP = NT // 2                    # 2 channel k-tile pairs (DoubleRow)
MT = HW // P                    # 8 token partition-tiles
MP = MT // 2                    # 4 token k-tile pairs
FCH = 512                       # output free-dim chunk (one PSUM bank fp32)
NCH = HW // FCH                 # 2 free chunks per 1024
F32 = mybir.dt.float32
F8 = mybir.dt.float8e4
DR = mybir.MatmulPerfMode.DoubleRow
WSC = 8.0                       # host weight prescale (fp8 subnormal lift)
SCALE = 1.0 / (WSC * math.sqrt(C))
ESHIFT = -2.0                   # exp shift: keeps exp(s) under e4m3 max 240

ACT_EXP = mybir.ActivationFunctionType.Exp
ACT_LN = mybir.ActivationFunctionType.Ln
ACT_COPY = mybir.ActivationFunctionType.Copy
OP_ADD = mybir.AluOpType.add
OP_MULT = mybir.AluOpType.mult

LAST_EXEC_NS = None
_CACHED = {}


def _build_nc(fused):
    from contextlib import ExitStack

    esc = 1.0 / ((WSC if fused else 1.0) * math.sqrt(C))
    nc = bacc.Bacc("TRN2", target_bir_lowering=False, debug=False)

    x_d = nc.dram_tensor("x", [BL, C, HW], F32, kind="ExternalInput").ap()
    # paired fp8 weights: [p, k, i, o] = w.T[(2k+i)*128+p, o]
    # fused mode: the wq8 slot carries 8*G^T pairs for Z = G h (G = wk^T wq)
    wq8_d = nc.dram_tensor("wq8", [P, KP, 2, C], F8, kind="ExternalInput").ap()
    wk8_d = nc.dram_tensor("wk8", [P, KP, 2, C], F8, kind="ExternalInput").ap()
    # wv8 slot carries 8*Wvo^T pairs, Wvo = wo @ wv
    wv8_d = nc.dram_tensor("wv8", [P, KP, 2, C], F8, kind="ExternalInput").ap()
    e8_d = nc.dram_tensor("e8", [P, 2, P], F8, kind="ExternalInput").ap()
    # cols pack: [p, j, t]: j = 0 gw, 1 gb, 2 bq, 3 bk, 4 boP
    cols_d = nc.dram_tensor("cols", [P, 5, NT], F32, kind="ExternalInput").ap()
    gm_d = nc.dram_tensor("gm", [P, NT, NGRP], F32, kind="ExternalInput").ap()
    gmt_d = nc.dram_tensor("gmt", [NGRP, NT, P], F32, kind="ExternalInput").ap()
    y_d = nc.dram_tensor("y", [BL, C, HW], F32, kind="ExternalOutput").ap()

    x_r = x_d.rearrange("b (t p) n -> b p t n", p=P)
    y_r = y_d.rearrange("b (t p) n -> b t p n", p=P)

    ib = lambda k, d: int(os.environ.get(k, d))  # buf-count knobs for tuning
    with tile.TileContext(nc) as tc, ExitStack() as ctx:
        pool = lambda name, bufs, space="SBUF": ctx.enter_context(
            tc.tile_pool(name=name, bufs=bufs, space=space)
        )
        p_const = pool("const", 1)
        p_x = pool("x", ib("BUF_X", 3))
        p_X = pool("X", ib("BUF_XN", 4))
        p_z = pool("z", 4)
        p_kt = pool("kt", 4)
        p_v = pool("v", 2)
        p_e = pool("exp", ib("BUF_EXP", 8))
        p_recip = pool("recip", 2)
        p_out = pool("out", ib("BUF_OUT", 3))
        p_small = pool("small", 4)
        psum = pool("psum", ib("BUF_PSUM", 8), space="PSUM")

        def ps_tile(name, parts=P, free=FCH):
            return psum.tile([parts, free], F32, tag="u", name=name)

        # ---- x loads: the DMA subsystem has ~9 completion semaphores, so
        # each outstanding dma_start is a scarce resource. One big 3D-AP
        # push per image ([p, t, 4KB]); image 0 uses two half pushes so
        # bn_stats starts on tiles 0-1 while 2-3 are still in flight ----
        def emit_x(b, halves=False):
            if halves:
                parts = []
                for h in range(2):
                    xh = p_x.tile(
                        [P, 2, HW], F32, tag="xh", bufs=2, name=f"x_{b}_{h}"
                    )
                    nc.sync.dma_start(out=xh[:], in_=x_r[b][:, 2 * h : 2 * h + 2])
                    parts.append(xh)
                return [parts[t // 2][:, t % 2] for t in range(NT)]
            xb = p_x.tile([P, NT, HW], F32, tag="x", name=f"x_{b}")
            nc.sync.dma_start(out=xb[:], in_=x_r[b])
            return [xb[:, t] for t in range(NT)]

        xt0 = emit_x(0, halves=True)

        # ---- fp8 weights + consts ride the scalar HWDGE queue so image-0
        # x owns the sync queue's full issue slot sequence ----
        def load_w8(dram, tag, eng):
            t = p_const.tile([P, KP, 2, C], F8, tag=tag)
            eng.dma_start(out=t[:], in_=dram)
            return t

        # gm first: the GN group-reduce needs it before anything else
        gm_sb = p_const.tile([P, NT, NGRP], F32, tag="gm")
        nc.scalar.dma_start(out=gm_sb[:], in_=gm_d)
        wq8 = load_w8(wq8_d, "wq8", nc.scalar)   # Z weights (8G pairs) fused
        wv8 = load_w8(wv8_d, "wv8", nc.scalar)   # 8*Wvo pairs
        wk8 = None if fused else load_w8(wk8_d, "wk8", nc.scalar)

        gmt_sb = p_const.tile([NGRP, NT, P], F32, tag="gmt")
        nc.scalar.dma_start(out=gmt_sb[:], in_=gmt_d)
        cols_sb = p_const.tile([P, 5, NT], F32, tag="cols")
        nc.scalar.dma_start(out=cols_sb[:], in_=cols_d)
        e8_sb = p_const.tile([P, 2, P], F8, tag="e8")
        nc.scalar.dma_start(out=e8_sb[:], in_=e8_d)
        eps_sb = p_const.tile([P, 1], F32, tag="eps")
        nc.vector.memset(eps_sb[:], EPS)
        esh_sb = p_const.tile([P, 1], F32, tag="esh")
        nc.vector.memset(esh_sb[:], ESHIFT)

        gw_sb = cols_sb[:, 0, :]
        gb_sb = cols_sb[:, 1, :]
        bq_sb = cols_sb[:, 2, :]
        bk_sb = cols_sb[:, 3, :]
        boP_sb = cols_sb[:, 4, :]

        def emit_gn_stats(b, xt):
            """DVE-only per-tile stats: stat2 = [mean, var + mean^2].
            bn_stats is capped at 512 free, so 2 chunks per tile."""
            stat2s = []
            for t in range(NT):
                st = p_small.tile([P, NCH, 6], F32, tag="bnst")
                for i in range(NCH):
                    nc.vector.bn_stats(
                        out=st[:, i, :], in_=xt[t][:, i * FCH : (i + 1) * FCH]
                    )
                stat2 = p_small.tile([P, 2], F32, tag="stat2", name=f"stat2_{b}_{t}")
                nc.vector.bn_aggr(out=stat2[:], in_=st[:])
                # var += mean^2 in one fused op: (mean * mean) + var
                nc.vector.scalar_tensor_tensor(
                    out=stat2[:, 1:2], in0=stat2[:, 0:1], scalar=stat2[:, 0:1],
                    in1=stat2[:, 1:2], op0=OP_MULT, op1=OP_ADD,
                )
                stat2s.append(stat2)
            return xt, stat2s

        def emit_gn_reduce(b, state):
            """Group-reduce via PE; rstd = rsqrt(var+eps). Fused mode
            returns gmr = [rstd, mean*rstd] so the broadcast feeds the
            affine x*a - b directly (host guarantees gw==1, gb==0)."""
            xt, stat2s = state
            psg = ps_tile(f"psg_{b}", parts=NGRP, free=2)
            for t in range(NT):
                nc.tensor.matmul(
                    psg[:], gm_sb[:, t, :], stat2s[t][:],
                    start=(t == 0), stop=(t == NT - 1),
                )
            # gmr: [32 groups, (mean, rstd)]; fused keeps -mean so every
            # downstream op stays on the fast (MULT, ADD) ucode path —
            # subtract variants of tensor_scalar/STT are ~10-70x slower
            gmr = p_small.tile([NGRP, 2], F32, tag="gmr")
            nc.vector.tensor_scalar_mul(
                gmr[:, 0:1], psg[:, 0:1], (-1.0 if fused else 1.0) / GS
            )
            e2g = p_small.tile([NGRP, 1], F32, tag="e2g")
            nc.vector.tensor_scalar_mul(e2g[:], psg[:, 1:2], 1.0 / GS)
            m2g = p_small.tile([NGRP, 1], F32, tag="m2g")
            nc.vector.tensor_mul(m2g[:], gmr[:, 0:1], gmr[:, 0:1])
            if fused:
                # DVE-only rsqrt: group var is ~1 for standardized x (host
                # checks and falls back otherwise), so Newton from y0 =
                # 1.5 - 0.5*ve converges to <1e-6 for var in [0.5, 2].
                # Avoids ACT act-table loads entirely. eps = 1e-5 is
                # dropped: var ~ 1 so it shifts rstd by ~5e-6 relative.
                ve = p_small.tile([NGRP, 1], F32, tag="ve")
                nc.vector.tensor_scalar(
                    out=ve[:], in0=m2g[:], scalar1=-1.0, scalar2=e2g[:],
                    op0=OP_MULT, op1=OP_ADD,
                )
                y = p_small.tile([NGRP, 1], F32, tag="nwy")
                nc.vector.tensor_scalar(
                    out=y[:], in0=ve[:], scalar1=-0.5, scalar2=1.5,
                    op0=OP_MULT, op1=OP_ADD,
                )
                t1 = p_small.tile([NGRP, 1], F32, tag="nwt")
                nc.vector.tensor_mul(t1[:], ve[:], y[:])
                nc.vector.tensor_mul(t1[:], t1[:], y[:])
                nc.vector.tensor_scalar(
                    out=t1[:], in0=t1[:], scalar1=-0.5, scalar2=1.5,
                    op0=OP_MULT, op1=OP_ADD,
                )
                # pack gmr2 = [rstd, -mean*rstd] for the affine x*a + b
                gmr2 = p_small.tile([NGRP, 2], F32, tag="gmr2")
                nc.vector.tensor_mul(gmr2[:, 0:1], y[:], t1[:])
                nc.vector.tensor_mul(gmr2[:, 1:2], gmr[:, 0:1], gmr2[:, 0:1])
                return xt, gmr2
            else:
                varg = p_small.tile([NGRP, 1], F32, tag="varg")
                nc.vector.tensor_sub(varg[:], e2g[:], m2g[:])
                lng = p_small.tile([NGRP, 1], F32, tag="lng")
                nc.scalar.activation(
                    out=lng[:], in_=varg[:], func=ACT_LN, bias=eps_sb[0:NGRP, :]
                )
                nc.scalar.activation(
                    out=gmr[:, 1:2], in_=lng[:], func=ACT_EXP, scale=-0.5
                )
            return xt, gmr

        def emit_gn_norm(b, state, head=False):
            """Broadcast group stats to channels, apply affine -> fp8 pairs.
            Head image alternates gpsimd/DVE per tile for latency; steady
            state keeps it all on gpsimd (DVE runs the evacs). Fused mode
            broadcasts [rstd, mean*rstd] into one PSUM bank, copies once to
            SBUF, and the affine is x*a - b (gw==1, gb==0 host-checked)."""
            xt, gmr = state
            X8 = [
                p_X.tile([P, 2, HW], F8, tag=f"X{k}", name=f"X_{b}_{k}")
                for k in range(KP)
            ]
            if fused:
                psb = ps_tile(f"psb_{b}", free=NT * 2)
                for t in range(NT):
                    nc.tensor.matmul(
                        psb[:, 2 * t : 2 * t + 2], gmt_sb[:, t, :], gmr[:],
                        start=True, stop=True,
                    )
                ab = p_small.tile([P, NT, 2], F32, tag="ab")
                nc.vector.tensor_copy(out=ab[:], in_=psb[:])
                for t in range(NT):
                    eng = (nc.vector if t % 2 else nc.gpsimd) if head else nc.gpsimd
                    eng.tensor_scalar(
                        out=X8[t // 2][:, t % 2, :], in0=xt[t][:],
                        scalar1=ab[:, t, 0:1], scalar2=ab[:, t, 1:2],
                        op0=OP_MULT, op1=OP_ADD,
                    )
                return xt, X8
            for t in range(NT):
                psb = ps_tile(f"psb_{b}_{t}", free=2)
                nc.tensor.matmul(psb[:], gmt_sb[:, t, :], gmr[:], start=True, stop=True)
                acol = p_small.tile([P, 1], F32, tag="acol")
                nc.vector.tensor_mul(acol[:], psb[:, 1:2], gw_sb[:, t : t + 1])
                tmb = p_small.tile([P, 1], F32, tag="tmb")
                nc.vector.tensor_mul(tmb[:], psb[:, 0:1], acol[:])
                bcol = p_small.tile([P, 1], F32, tag="bcol")
                nc.vector.tensor_sub(bcol[:], gb_sb[:, t : t + 1], tmb[:])
                eng = (nc.vector if t % 2 else nc.gpsimd) if head else nc.gpsimd
                eng.tensor_scalar(
                    out=X8[t // 2][:, t % 2, :], in0=xt[t][:],
                    scalar1=acol[:], scalar2=bcol[:], op0=OP_MULT, op1=OP_ADD,
                )
            return xt, X8

        xt_pre = {1: emit_x(1)} if BL > 1 else {}
        gn_state = emit_gn_norm(
            0, emit_gn_reduce(0, emit_gn_stats(0, xt0)), head=True
        )

        # ---- per-image heavy phases ----
        for b in range(BL):
            xt, X8 = gn_state
            # prefetch x two images ahead so bn_stats never waits on DMA
            if b + 2 < BL:
                xt_pre[b + 2] = emit_x(b + 2)

            # channel-major projection [C, HW]; fused Z evacs ride on ACT
            # (Copy needs no act table), biased Q/K evacs use DVE
            def proj_cm(w8, bias_sb, out_pool, bname):
                dst = [
                    out_pool.tile(
                        [P, 2, HW], F8, tag=f"{bname}{k}", name=f"{bname}_{b}_{k}"
                    )
                    for k in range(KP)
                ]
                for ot in range(NT):
                    for nch in range(NCH):
                        ps = ps_tile(f"ps_{bname}_{b}_{ot}_{nch}")
                        for k in range(KP):
                            nc.tensor.matmul(
                                ps[:],
                                w8[:, k, :, ot * P : (ot + 1) * P],
                                X8[k][:, :, nch * FCH : (nch + 1) * FCH],
                                start=(k == 0),
                                stop=(k == KP - 1),
                                perf_mode=DR,
                            )
                        dslice = dst[ot // 2][:, ot % 2, nch * FCH : (nch + 1) * FCH]
                        if bias_sb is None:
                            nc.scalar.activation(
                                out=dslice, in_=ps[:], func=ACT_COPY, bias=0.0
                            )
                        else:
                            nc.vector.tensor_scalar(
                                out=dslice, in0=ps[:],
                                scalar1=bias_sb[:, ot : ot + 1], scalar2=None,
                                op0=OP_ADD,
                            )
                return dst

            if fused:
                Z8 = proj_cm(wq8, None, p_z, "z")
                s_stat, s_mov = X8, Z8
            else:
                QT8 = proj_cm(wq8, bq_sb, p_z, "q")
                KT8 = proj_cm(wk8, bk_sb, p_kt, "k")
                s_stat, s_mov = KT8, QT8

            # scores S^T[m, n] -> exp (fp8, shifted); per-m-pair tiles so PV
            # can start before the whole phase has evacuated
            E8 = [
                p_e.tile([P, 2, HW], F8, tag="exp", name=f"e_{b}_{j}")
                for j in range(MP)
            ]
            for mt in range(MT):
                for nch in range(NCH):
                    psS = ps_tile(f"ps_s_{b}_{mt}_{nch}")
                    for k in range(KP):
                        nc.tensor.matmul(
                            psS[:],
                            s_stat[k][:, :, mt * P : (mt + 1) * P],
                            s_mov[k][:, :, nch * FCH : (nch + 1) * FCH],
                            start=(k == 0),
                            stop=(k == KP - 1),
                            perf_mode=DR,
                        )
                    nc.scalar.activation(
                        out=E8[mt // 2][:, mt % 2, nch * FCH : (nch + 1) * FCH],
                        in_=psS[:], func=ACT_EXP, scale=esc, bias=esh_sb[:],
                    )

            # GN(b+1) stats+reduce: emitted after S so they queue on DVE
            # behind nothing (DVE idles during Z/S); the newton chain's
            # latency hides under the V matmuls
            reduce_next = (
                emit_gn_reduce(b + 1, emit_gn_stats(b + 1, xt_pre[b + 1]))
                if b + 1 < BL
                else None
            )

            # V' projection (token-major pairs); emitted after S so the PE
            # stays busy while ACT drains the exp backlog PV depends on
            V8 = p_v.tile([P, MP, 2, C], F8, tag="v", name=f"v_{b}")
            for mt in range(MT):
                ps = ps_tile(f"ps_v_{b}_{mt}")
                for k in range(KP):
                    nc.tensor.matmul(
                        ps[:],
                        X8[k][:, :, mt * P : (mt + 1) * P],
                        wv8[:, k, :, :],
                        start=(k == 0),
                        stop=(k == KP - 1),
                        perf_mode=DR,
                    )
                nc.vector.tensor_copy(out=V8[:, mt // 2, mt % 2, :], in_=ps[:])

            # GN(b+1) broadcast+affine: psb after V so it never stalls on the
            # newton chain; gpsimd affines overlap PV + evac
            gn_next = (
                emit_gn_norm(b + 1, reduce_next)
                if reduce_next is not None
                else None
            )

            # A^T[c, n] accumulated over m-pairs; y = A*recip + x fused into
            # the evac (output projection already folded into V'). colsum
            # uses an 8.0-valued stationary to cancel the host Wvo x8 scale.
            recip = p_recip.tile([P, HW], F32, tag="recip", name=f"recip_{b}")
            last = b == BL - 1
            # colsum first: recip is ready as soon as the first psA lands
            for nch in range(NCH):
                psc_t = ps_tile(f"psc_{b}_{nch}")
                for j in range(MP):
                    nc.tensor.matmul(
                        psc_t[:],
                        e8_sb[:],
                        E8[j][:, :, nch * FCH : (nch + 1) * FCH],
                        start=(j == 0),
                        stop=(j == MP - 1),
                        perf_mode=DR,
                    )
                nc.vector.reciprocal_approx_fast(
                    out=recip[:, nch * FCH : (nch + 1) * FCH],
                    in_=psc_t[:],
                )
            for c2 in range(NT):
                tm = p_out.tile([P, HW], F32, tag="tm", name=f"tm_{b}_{c2}")
                oy = p_out.tile([P, HW], F32, tag="oy", name=f"oy_{b}_{c2}")
                for nch in range(NCH):
                    ps_at = ps_tile(f"ps_a_{b}_{c2}_{nch}")
                    for j in range(MP):
                        nc.tensor.matmul(
                            ps_at[:],
                            V8[:, j, :, c2 * P : (c2 + 1) * P],
                            E8[j][:, :, nch * FCH : (nch + 1) * FCH],
                            start=(j == 0),
                            stop=(j == MP - 1),
                            perf_mode=DR,
                        )
                    nc.vector.tensor_mul(
                        tm[:, nch * FCH : (nch + 1) * FCH],
                        ps_at[:],
                        recip[:, nch * FCH : (nch + 1) * FCH],
                    )
                if fused:
                    # residual add (never in-place: aliased DVE adds hit a
                    # slow ucode path); gpsimd steady-state so DVE keeps
                    # stats/cast latency, DVE on the last image's drain
                    eng = nc.vector if last else nc.gpsimd
                    eng.tensor_add(oy[:], tm[:], xt[c2][:])
                else:
                    for nch in range(NCH):
                        sl = slice(nch * FCH, (nch + 1) * FCH)
                        nc.vector.scalar_tensor_tensor(
                            out=oy[:, sl], in0=tm[:, sl],
                            scalar=boP_sb[:, c2 : c2 + 1],
                            in1=xt[c2][:, sl], op0=OP_ADD, op1=OP_ADD,
                        )
                # y store: one [128, 4KB] push; last image alternates queues
                eng = nc.scalar if (last and c2 % 2) else nc.sync
                eng.dma_start(out=y_r[b, c2], in_=oy[:])

            if gn_next is not None:
                gn_state = gn_next

    nc.compile()
    return nc


def _host_inputs(fused, x, gn_scale, gn_bias, wq, bq, wk, bk, wv, bv, wo, bo):
    f = lambda a: np.ascontiguousarray(np.asarray(a, dtype=np.float32))
    x = f(x).reshape(B, C, HW)
    boP = f(bo) + f(wo) @ f(bv)

    def pair8(wT):
        # [p, k, i, o] = wT[(2k+i)*128+p, o], quantized to e4m3
        t = np.ascontiguousarray(wT).reshape(KP, 2, P, C).transpose(2, 0, 1, 3)
        return np.ascontiguousarray(
            np.clip(t, -240, 240).astype(ml_dtypes.float8_e4m3)
        )

    if fused:
        G = f(wk).T @ f(wq)          # S^T[m,n] = h_m^T G h_n
        wq8 = pair8(WSC * G.T)       # stationary pairs of 8G
        wk8 = np.zeros((P, KP, 2, C), ml_dtypes.float8_e4m3)
    else:
        wq8 = pair8(f(wq).T)
        wk8 = pair8(f(wk).T)

    Wvo = f(wo) @ f(wv)              # out = P (h Wvo^T) + boP
    wv8 = pair8((WSC if fused else 1.0) * Wvo.T)

    gm = np.zeros((P, NT, NGRP), np.float32)
    gmt = np.zeros((NGRP, NT, P), np.float32)
    for t in range(NT):
        for p in range(P):
            g = (t * P + p) // GS
            gm[p, t, g] = 1.0
            gmt[g, t, p] = 1.0
    e8 = np.full((P, 2, P), WSC if fused else 1.0, ml_dtypes.float8_e4m3)

    colv = lambda a: f(a).reshape(NT, P).T            # [(t p)] -> [p, t]
    cols = np.ascontiguousarray(
        np.stack(
            [colv(gn_scale), colv(gn_bias), colv(bq), colv(bk), colv(boP)],
            axis=1,
        )
    )

    shared = {
        "wq8": wq8,
        "wk8": wk8,
        "wv8": wv8,
        "cols": cols,
        "gm": gm, "gmt": gmt, "e8": e8,
    }
    in_maps = []
    for i in range(N_CORES):
        m = dict(shared)
        m["x"] = np.ascontiguousarray(x[i * BL : (i + 1) * BL])
        in_maps.append(m)
    return in_maps


def kernel(x, gn_scale, gn_bias, wq, bq, wk, bk, wv, bv, wo, bo):
    global LAST_EXEC_NS
    assert x.shape == (B, C, H, W)
    # Z-fusion drops bq into softmax-invariant terms. The fused GN rstd uses
    # Newton-from-1 which needs roughly unit group variance, so oddly-scaled
    # x routes to the general kernel. In the fused kernel the exp scale
    # assumes the 8x host weight prescale; non-fused keeps unscaled weights.
    xs = np.asarray(x, np.float32).ravel()[::1031][:8192]
    fused = (
        not np.any(np.asarray(bq))
        and np.all(np.asarray(gn_scale) == 1.0)
        and not np.any(np.asarray(gn_bias))
        and 0.5 < float(xs.var()) < 2.0
    )
    if fused not in _CACHED:
        _CACHED[fused] = _build_nc(fused)
    in_maps = _host_inputs(
        fused, x, gn_scale, gn_bias, wq, bq, wk, bk, wv, bv, wo, bo
    )
    trace = os.environ.get("ATT_TRACE", "0") == "1"
    if not trace:
        # the NTFF trace path needs antenv.axon_hooks (shimmed only by our
        # test harness); make sure a stray BASS_TRACE can't drag us into it
        os.environ["BASS_NEVER_TRACE"] = "1"
    else:
        os.environ.pop("BASS_NEVER_TRACE", None)
    kwargs = {}
    tdir = os.environ.get("ATT_TRACE_DIR")
    if tdir:
        kwargs["tmpdir"] = tdir
    res = run_bass_kernel_spmd(
        _CACHED[fused], in_maps, core_ids=list(range(N_CORES)), trace=trace, **kwargs
    )
    LAST_EXEC_NS = res.exec_time_ns
    y = np.concatenate([res.results[i]["y"] for i in range(N_CORES)], axis=0)
    return y.reshape(B, C, H, W).astype(np.float32)
